# revision 31
# baseline (speedup 1.0000x reference)
"""ByteAddFFN Trainium2 kernel builder (single-core SPMD program).

Per item, per byte j (faithful to reference.py):
  la/ha = low/high nibble logits of a (segment sums of the 256-wide byte vec);
  E_lo[16x+y] = exp(100*(la[x] + lb[y] - mla - mlb)): the logit sums are done
      on the PE via one-hot matmuls in bf16 with an exact 3-way mantissa
      split (v = v1+v2+v3, each bf16; PSUM accumulates to fp32), the shift
      is folded into the b-side logits, and ACT applies exp from PSUM;
  [U | S0 S1 S0' S1' | ZZ] = TABLE.T @ E  (plain-f32 PE matmul, item-major);
  carry chain over 8 nibble stages (tiny per-item ops), with the 1/ZZ
      normalizer folded into the carries;
  out = softmax16(100*softmax16(100*s_raw_hi)) (x) same for lo  (the
      reference's nested softmax), second max-sub replaced by a fixed -100
      shift since softmax outputs live in [0,1].
"""
import numpy as np
import ml_dtypes

import concourse.bass as bass
import concourse.mybir as mybir

F32 = mybir.dt.float32
F32R = mybir.dt.float32r
BF16 = mybir.dt.bfloat16
ALU = mybir.AluOpType
AX = mybir.AxisListType
ACT = mybir.ActivationFunctionType
BF = ml_dtypes.bfloat16


def build_consts2():
    """Extended table for the reoriented (m-major) U matmul.

    c_table2 [128, 2, 64] f32: row r of chunk c corresponds to m = 128c+r
    (t = (m>>4)+(m&15)); cols follow the UL1 layout directly:
      0:16  U      (one-hot at t%16)
      16:20 S0 S1 S0' S1'
      20    ZZ
      21:37 rotU   (one-hot at (t+1)%16)
      37:64 zero padding (so two 64-row outputs pack one PSUM tile)
    """
    t2 = np.zeros((128, 2, 64), np.float32)
    for c in range(2):
        for r in range(128):
            t = ((128 * c + r) >> 4) + ((128 * c + r) & 15)
            t2[r, c, t % 16] = 1.0
            t2[r, c, 16 + (1 if t >= 16 else 0)] = 1.0
            t2[r, c, 18 + (1 if t >= 15 else 0)] = 1.0
            t2[r, c, 20] = 1.0
            t2[r, c, 21 + ((t + 1) % 16)] = 1.0
    return t2


def build_consts():
    ident = np.eye(128, dtype=BF)
    # REPLB [96, 256]: col m = 16x+y; ones at rows {x, 16+y} for each of the
    # three 32-row split blocks.
    replb = np.zeros((96, 256), BF)
    for m in range(256):
        for s in range(3):
            replb[32 * s + (m >> 4), m] = 1.0
            replb[32 * s + 16 + (m & 15), m] = 1.0
    # TABLE [128, 2, 21]: chunk c row r -> global xy = 128c + r
    table = np.zeros((128, 2, 21), np.float32)
    for c in range(2):
        for r in range(128):
            t = ((128 * c + r) >> 4) + ((128 * c + r) & 15)
            table[r, c, t % 16] = 1.0                      # U
            table[r, c, 16 + (1 if t >= 16 else 0)] = 1.0  # S0/S1
            table[r, c, 18 + (1 if t >= 15 else 0)] = 1.0  # S0'/S1'
            table[r, c, 20] = 1.0                          # ZZ
    return {"c_ident": ident, "c_repl": replb, "c_table": table,
            "c_table2": build_consts2()}


# UL1 column layout
UC_U = 0      # 0:16   U
UC_S = 16     # 16:20  S0 S1 S0' S1'
UC_ZZ = 20    # 20     ZZ
UC_ROT = 21   # 21:37  rotU (filled by global copies: [U15, U0..U14])
UC_N = 37


def build_kernel(nc, tc, ctx, M, F=2, gp_prefold=True, gp_shift=True,
                 out_gp_frac=0.3, reps=1, n_tails=2, hi_gp=0, esb_bf16=False,
                 u_swap=False, chain_gp=False, misc_gp=False):
    SLOTS = M // 128
    NST = SLOTS // F
    assert SLOTS % F == 0 and M % 128 == 0
    N = F * 128

    a_d = nc.dram_tensor("a", (M, 4, 256), F32, kind="ExternalInput")
    b_d = nc.dram_tensor("b", (M, 4, 256), F32, kind="ExternalInput")
    id_d = nc.dram_tensor("c_ident", (128, 128), BF16, kind="ExternalInput")
    repl_d = nc.dram_tensor("c_repl", (96, 256), BF16, kind="ExternalInput")
    tab_d = nc.dram_tensor("c_table", (128, 2, 21), F32, kind="ExternalInput")
    tab2_d = nc.dram_tensor("c_table2", (128, 2, 64), F32,
                            kind="ExternalInput")
    o_d = nc.dram_tensor("out", (M, 4, 256), F32, kind="ExternalOutput")

    a_v = a_d.ap().rearrange("(p t) b c -> p t (b c)", p=128)
    b_v = b_d.ap().rearrange("(p t) b c -> p t (b c)", p=128)
    o_v = o_d.ap().rearrange("(p t) b c -> p t (b c)", p=128)

    cpool = ctx.enter_context(tc.tile_pool(name="consts", bufs=1))
    abp = ctx.enter_context(tc.tile_pool(name="ab", bufs=2))
    stp = ctx.enter_context(
        tc.tile_pool(name="st", bufs=2 if (u_swap or hi_gp) else 3))
    qp = ctx.enter_context(tc.tile_pool(name="q", bufs=2))
    qhp = ctx.enter_context(tc.tile_pool(name="qh", bufs=1))
    rhsp = ctx.enter_context(
        tc.tile_pool(name="rhs", bufs=2 if (u_swap or hi_gp) else 3))
    esbp = ctx.enter_context(tc.tile_pool(name="esb", bufs=2))
    persist = ctx.enter_context(tc.tile_pool(name="persist", bufs=1))
    outp = ctx.enter_context(tc.tile_pool(name="outp", bufs=2))
    smallp = ctx.enter_context(tc.tile_pool(name="small", bufs=2))
    tpp = ctx.enter_context(
        tc.tile_pool(name="tp", bufs=1 if u_swap else 2, space="PSUM"))
    epp = ctx.enter_context(tc.tile_pool(name="ep", bufs=2, space="PSUM"))
    upp = ctx.enter_context(tc.tile_pool(name="up", bufs=1, space="PSUM"))
    usbp = ctx.enter_context(tc.tile_pool(name="usb", bufs=1))

    ident = cpool.tile([128, 128], BF16)
    nc.sync.dma_start(ident[:], id_d.ap())
    repl = cpool.tile([96, 256], BF16)
    nc.sync.dma_start(repl[:], repl_d.ap())
    tabl = cpool.tile([128, 2, 21], F32)
    nc.sync.dma_start(tabl[:], tab_d.ap())
    if esb_bf16:
        tablb = cpool.tile([128, 2, 21], BF16)
        nc.vector.tensor_copy(tablb[:], tabl[:])
        tabl = tablb
    bm100 = cpool.tile([128, 1], F32)
    nc.gpsimd.memset(bm100[:], -100.0)
    if u_swap:
        tab2f = cpool.tile([128, 2, 64], F32)
        nc.sync.dma_start(tab2f[:], tab2_d.ap())
        tab2 = cpool.tile([128, 2, 64], F32R)
        nc.vector.tensor_copy(tab2[:], tab2f[:])
        identf = cpool.tile([128, 128], F32)
        nc.vector.tensor_copy(identf[:], ident[:])

    UL1 = persist.tile([128, SLOTS, 8, UC_N], F32)
    SRAW = persist.tile([128, SLOTS, 8, 16], F32)
    EHLO = persist.tile([128, SLOTS, 4, 16], F32)
    EHHI = persist.tile([128, SLOTS, 4, 16], F32)
    ZETA = persist.tile([128, SLOTS, 8], F32)
    CARR = persist.tile([128, SLOTS, 2], F32)
    TMPC = persist.tile([128, SLOTS, 16], F32)

    # rotU fill + zeta + chain + final, per slot-range (allows the tail of
    # one half to overlap the supertile streaming of the next half)
    def tail_phase(t0, t1):
        nsl = t1 - t0
        u3 = UL1[:, t0:t1, :, :].rearrange("p t k c -> p (t k) c")
        if not u_swap:
            nc.scalar.copy(u3[:, :, UC_ROT:UC_ROT + 1], u3[:, :, 15:16])
            nc.scalar.copy(u3[:, :, UC_ROT + 1:UC_ROT + 16], u3[:, :, 0:15])

        ZET = ZETA[:, t0:t1, :]
        CAR = CARR[:, t0:t1, :]
        TMP = TMPC[:, t0:t1, :]
        nc.vector.reciprocal(ZET[:], UL1[:, t0:t1, :, UC_ZZ])
        (nc.gpsimd if chain_gp else nc.vector).tensor_copy(
            CAR[:], ZET[:, :, 0:1].broadcast_to((128, nsl, 2)))

        for k in range(8):
            nc.gpsimd.tensor_tensor(
                SRAW[:, t0:t1, k, :], UL1[:, t0:t1, k, UC_U:UC_U + 16],
                CAR[:, :, 0:1].broadcast_to((128, nsl, 16)), ALU.mult)
            nc.gpsimd.tensor_tensor(
                TMP[:], UL1[:, t0:t1, k, UC_ROT:UC_ROT + 16],
                CAR[:, :, 1:2].broadcast_to((128, nsl, 16)), ALU.mult)
            nc.gpsimd.tensor_tensor(
                SRAW[:, t0:t1, k, :], SRAW[:, t0:t1, k, :], TMP[:], ALU.add)

            if k == 7:
                break
            t4t = smallp.tile([128, nsl, 2, 2], F32, tag="ch4")
            t1t = smallp.tile([128, nsl, 2], F32, tag="ch1")
            ceng = nc.gpsimd if chain_gp else nc.vector
            # UL1 S-cols are [S0 S1 S0' S1']; pair with (c0,c0,c1,c1)
            ceng.tensor_tensor(
                t4t[:], UL1[:, t0:t1, k, UC_S:UC_S + 4].rearrange(
                    "p t (a b) -> p t a b", a=2),
                CAR[:, :, :].unsqueeze(3).broadcast_to((128, nsl, 2, 2)),
                ALU.mult)
            ceng.tensor_tensor(
                t1t[:], t4t[:, :, 0, :], t4t[:, :, 1, :], ALU.add)
            nmx = smallp.tile([128, nsl, 1], F32, tag="chm")
            nc.vector.tensor_reduce(nmx[:], t1t[:], AX.X, ALU.max, negate=True)
            ceng.tensor_tensor(
                t1t[:], t1t[:], nmx[:].broadcast_to((128, nsl, 2)), ALU.add)
            e2 = smallp.tile([128, nsl, 2], F32, tag="che")
            nc.scalar.activation(e2[:], t1t[:], ACT.Exp, scale=100.0)
            z2 = smallp.tile([128, nsl, 1], F32, tag="chz")
            nc.vector.tensor_reduce(z2[:], e2[:], AX.X, ALU.add)
            rz = smallp.tile([128, nsl, 1], F32, tag="chr")
            nc.vector.reciprocal(rz[:], z2[:])
            ceng.tensor_tensor(
                rz[:], rz[:], ZET[:, :, k + 1:k + 2], ALU.mult)
            ceng.tensor_tensor(
                CAR[:], e2[:], rz[:].broadcast_to((128, nsl, 2)), ALU.mult)

        # final nested softmax factors
        NMX16 = smallp.tile([128, nsl, 8], F32, tag="nmx16")
        srv = SRAW[:, t0:t1, :, :].rearrange("p t k c -> p (t k) c")
        nc.vector.tensor_reduce(NMX16[:], srv, AX.X, ALU.max, negate=True)
        nc.gpsimd.tensor_tensor(
            srv, srv,
            NMX16[:].rearrange("p t k -> p (t k)").unsqueeze(2).broadcast_to(
                (128, nsl * 8, 16)), ALU.add)
        sv = SRAW[:, t0:t1, :, :].rearrange("p t (j s) c -> p s (t j) c", s=2)
        elo = EHLO[:, t0:t1, :, :].rearrange("p t j c -> p (t j) c")
        ehi = EHHI[:, t0:t1, :, :].rearrange("p t j c -> p (t j) c")
        nc.scalar.activation(elo, sv[:, 0, :, :], ACT.Exp, scale=100.0)
        nc.scalar.activation(ehi, sv[:, 1, :, :], ACT.Exp, scale=100.0)
        ZLO = smallp.tile([128, nsl, 4], F32, tag="zlo")
        ZHI = smallp.tile([128, nsl, 4], F32, tag="zhi")
        # two-level normalize per side (sides kept separate: the combined
        # Z2lo*Z2hi normalizer can underflow f32)
        for (ev, Z, geng) in ((elo, ZLO, nc.vector), (ehi, ZHI, nc.gpsimd)):
            for _lvl in range(2):
                nc.vector.tensor_reduce(Z[:], ev, AX.X, ALU.add)
                nc.vector.reciprocal(Z[:], Z[:])
                geng.tensor_tensor(
                    ev, ev,
                    Z[:].rearrange("p t j -> p (t j)").unsqueeze(2)
                    .broadcast_to((128, nsl * 4, 16)), ALU.mult)
                if _lvl == 0:
                    nc.scalar.activation(
                        ev, ev, ACT.Exp, scale=100.0, bias=bm100[:])

        n_gp = int(round(out_gp_frac * nsl / 2.0))
        for tt in range(nsl // 2):
            t = t0 // 2 + tt
            ot = outp.tile([128, 2, 4, 16, 16], F32)
            eng = nc.gpsimd if tt < n_gp else nc.vector
            eng.tensor_tensor(
                ot[:].rearrange("p t j h l -> p (t j) h l"),
                EHHI[:, 2 * t:2 * t + 2, :, :].rearrange(
                    "p t j c -> p (t j) c").unsqueeze(3).broadcast_to(
                    (128, 8, 16, 16)),
                EHLO[:, 2 * t:2 * t + 2, :, :].rearrange(
                    "p t j c -> p (t j) c").unsqueeze(2).broadcast_to(
                    (128, 8, 16, 16)),
                ALU.mult)
            nc.sync.dma_start(
                o_v[:, 2 * t:2 * t + 2, :],
                ot[:].rearrange("p t j h l -> p t (j h l)"))


    # uneven tail boundaries: earlier (larger) tails overlap the supertile
    # stream; the last (exposed) tail is kept small. F-aligned.
    def _align(x):
        return (x // F) * F
    if SLOTS >= 16 and n_tails >= 3:
        tail_bounds = [0, _align(3 * SLOTS // 8), _align(6 * SLOTS // 8), SLOTS]
    elif SLOTS >= 4:
        tail_bounds = [0, _align(SLOTS * (n_tails - 1) // n_tails), SLOTS]
    else:
        tail_bounds = [0, SLOTS]
    tail_bounds = sorted(set(tail_bounds))

    for _rep in range(reps):
        # ---------------- supertile loop ----------------
        for s in range(NST):
            at = abp.tile([128, F, 4, 256], F32, tag="at")
            nc.sync.dma_start(at[:], a_v[:, s * F:(s + 1) * F, :])
            bt = abp.tile([128, F, 4, 256], F32, tag="bt")
            nc.sync.dma_start(bt[:], b_v[:, s * F:(s + 1) * F, :])

            st = stp.tile([128, F, 4, 64], F32, tag="stf")
            av = at[:].rearrange("p f b (h l) -> p (f b) h l", h=16, l=16)
            bv = bt[:].rearrange("p f b (h l) -> p (f b) h l", h=16, l=16)
            stv = st[:].rearrange("p f b c -> p (f b) c")

            for ti, (xv, lo_off, hi_off, qtag) in enumerate((
                    (av, 0, 32, "qa"), (bv, 16, 48, "qb"))):
                if ti < hi_gp:
                    qh = qhp.tile([128, F * 4, 16, 8], F32, tag=qtag + "h")
                    nc.gpsimd.tensor_tensor(
                        qh[:], xv[:, :, :, 0:8], xv[:, :, :, 8:16], ALU.add)
                    nc.vector.tensor_reduce(
                        stv[:, :, hi_off:hi_off + 16], qh[:], AX.X, ALU.add)
                else:
                    nc.vector.tensor_reduce(
                        stv[:, :, hi_off:hi_off + 16], xv, AX.X, ALU.add)
                if gp_prefold:
                    q = qp.tile([128, F * 4, 8, 16], F32, tag=qtag)
                    xw = xv.rearrange("p g h l -> p g (h l)").rearrange(
                        "p g (hp two l) -> p g hp two l", two=2, l=16)
                    nc.gpsimd.tensor_tensor(
                        q[:], xw[:, :, :, 0, :], xw[:, :, :, 1, :], ALU.add)
                    nc.vector.tensor_reduce(
                        stv[:, :, lo_off:lo_off + 16],
                        q[:].rearrange("p g hp l -> p g l hp"), AX.X, ALU.add)
                else:
                    nc.vector.tensor_reduce(
                        stv[:, :, lo_off:lo_off + 16],
                        xv.rearrange("p g h l -> p g l h"), AX.X, ALU.add)

            # negated maxes (one fused reduce over [la lb ha hb] 16-groups),
            # then nm[g, s] = nm_a[s] + nm_b[s], then one fused b-side shift
            nma = smallp.tile([128, F * 4, 4], F32, tag="nma")
            nm = smallp.tile([128, F * 4, 2], F32, tag="nm")
            nc.vector.tensor_reduce(
                nma[:], stv.rearrange("p g (q c) -> p g q c", q=4),
                AX.X, ALU.max, negate=True)
            (nc.gpsimd if misc_gp else nc.vector).tensor_tensor(
                nm[:], nma[:, :, 0::2], nma[:, :, 1::2], ALU.add)
            sh_eng = nc.gpsimd if gp_shift else nc.vector
            stsh = stv.rearrange("p g (s c) -> p g s c", s=2)[:, :, :, 16:32]
            sh_eng.tensor_tensor(
                stsh, stsh,
                nm[:].unsqueeze(3).broadcast_to((128, F * 4, 2, 16)), ALU.add)

            # exact 3-way bf16 split, both sides per op; casts on gpsimd
            stb = stp.tile([128, F, 4, 2, 96], BF16, tag="stb")
            sb = stb[:].rearrange("p f b s c -> p (f b) s c")
            vb = stv.rearrange("p g (s c) -> p g s c", s=2)
            r1 = smallp.tile([128, F * 4, 2, 32], F32, tag="r1")
            nc.gpsimd.tensor_copy(sb[:, :, :, 0:32], vb)
            nc.vector.tensor_tensor(r1[:], vb, sb[:, :, :, 0:32], ALU.subtract)
            nc.gpsimd.tensor_copy(sb[:, :, :, 32:64], r1[:])
            nc.vector.tensor_tensor(
                r1[:], r1[:], sb[:, :, :, 32:64], ALU.subtract)
            nc.gpsimd.tensor_copy(sb[:, :, :, 64:96], r1[:])

            # transpose each (f, j, side) [128, 96] -> [96, 128]; batched copy
            rhs = rhsp.tile([96, 2, 4, N], BF16, tag="rhs")
            for f in range(F):
                tp = tpp.tile([96, 2, 4, 128], BF16, tag="tp")
                for si in range(2):
                    for j in range(4):
                        nc.tensor.transpose(
                            tp[:, si, j, :], stb[:, f, j, si, :], ident[:])
                nc.scalar.copy(rhs[:, :, :, f * 128:(f + 1) * 128], tp[:])

            # E matmuls (bf16 K=96, exact via split) + exp from PSUM
            esb_dt = BF16 if esb_bf16 else (F32R if u_swap else F32)
            esb = esbp.tile([128, 2, 2, 2, 2, N], esb_dt)
            for j in range(4):
                ep = epp.tile([128, 2, 2, N], F32, tag="ep")
                for side in range(2):
                    for c in range(2):
                        nc.tensor.matmul(
                            ep[:, side, c, :],
                            repl[:, c * 128:(c + 1) * 128],
                            rhs[:, side, j, :])
                nc.scalar.activation(
                    esb[:, j >> 1, j & 1, :, :, :].rearrange(
                        "p s c n -> p (s c) n"),
                    ep[:].rearrange("p s c n -> p (s c) n"),
                    ACT.Exp, scale=100.0)

            if u_swap:
                # U matmuls reoriented: TABLE2 stationary (37 f32r cols),
                # E streams as f32r at N=256 -> m-major U [37pad64*2, 4, 256]
                # in PSUM; PE-transpose back to item-major.
                upm = upp.tile([128, 4, 256], F32, tag="upm")
                for j in range(4):
                    for side in range(2):
                        k = 2 * j + side
                        for c in range(2):
                            nc.tensor.matmul(
                                upm[64 * (k % 2):64 * (k % 2) + 64,
                                    k // 2, :],
                                tab2[:, c, :],
                                esb[:, j >> 1, j & 1, side, c, :],
                                start=(c == 0), stop=(c == 1))
                usb = usbp.tile([128, 4, 256], F32, tag="usb")
                nc.scalar.copy(usb[:], upm[:])
                for f in range(F):
                    tpu = upp.tile([128, 4, 2, 64], F32, tag="tpu")
                    for kk in range(4):
                        nc.tensor.transpose(
                            tpu[:, kk, :, :].rearrange("p a b -> p (a b)"),
                            usb[:, kk, f * 128:(f + 1) * 128],
                            identf[:])
                    nc.scalar.copy(
                        UL1[:, s * F + f, :, 0:UC_N],
                        tpu[:, :, :, 0:UC_N].rearrange(
                            "p kk half c -> p (kk half) c"))
            else:
                # U matmuls: plain f32, E chunk stationary -> item-major
                for f in range(F):
                    up = upp.tile([128, 8, 21], F32, tag="up")
                    for j in range(4):
                        for side in range(2):
                            k = 2 * j + side
                            for c in range(2):
                                nc.tensor.matmul(
                                    up[:, k, :],
                                    esb[:, j >> 1, j & 1, side, c,
                                        f * 128:(f + 1) * 128],
                                    tabl[:, c, :],
                                    start=(c == 0), stop=(c == 1))
                    nc.scalar.copy(UL1[:, s * F + f, :, 0:21], up[:])

            for qq in range(1, len(tail_bounds) - 1):
                if (s + 1) * F == tail_bounds[qq]:
                    tail_phase(tail_bounds[qq - 1], tail_bounds[qq])

        tail_phase(tail_bounds[-2], tail_bounds[-1])




# ======================================================================
# Runner: shard across 8 NeuronCores, compile once, execute via PJRT/axon
# ======================================================================
N_CORES = 8
_CACHE = {}

# Build-variant used by kernel() (the graded entry point).
DEFAULT_BK = {"hi_gp": 2, "chain_gp": True, "misc_gp": True,
              "n_tails": 2, "out_gp_frac": 0.45}


def _get_compiled(m_per_core, f=2, reps=1, **bk):
    key = (m_per_core, f, reps, tuple(sorted(bk.items())))
    if key not in _CACHE:
        from contextlib import ExitStack
        import concourse.bacc as bacc
        import concourse.tile as tile
        nc = bacc.Bacc("TRN2", target_bir_lowering=False, debug=False)
        with tile.TileContext(nc) as tc:
            with ExitStack() as ctx:
                build_kernel(nc, tc, ctx, m_per_core, F=f, reps=reps, **bk)
        nc.compile()
        _CACHE[key] = nc
    return _CACHE[key]


def kernel(a, b, b2n=None, n2b=None, add_table=None, carry_table=None,
           **_ignored):
    """Full-input entry point: a, b [32768, 4, 256] f32 -> out [32768, 4, 256].

    Shards the batch across the 8 visible NeuronCores (pure data parallel),
    runs the Bass kernel SPMD, and concatenates the per-core outputs.
    """
    from concourse.bass_utils import run_bass_kernel_spmd

    a = np.ascontiguousarray(np.asarray(a, dtype=np.float32))
    b = np.ascontiguousarray(np.asarray(b, dtype=np.float32))
    B = a.shape[0]
    assert B % N_CORES == 0, f"batch {B} not divisible by {N_CORES} cores"
    mpc = B // N_CORES

    nc = _get_compiled(mpc, **DEFAULT_BK)
    consts = build_consts()
    in_maps = []
    for c in range(N_CORES):
        m = {"a": a[c * mpc:(c + 1) * mpc], "b": b[c * mpc:(c + 1) * mpc]}
        m.update(consts)
        in_maps.append(m)
    res = run_bass_kernel_spmd(nc, in_maps, core_ids=list(range(N_CORES)))
    out = np.concatenate([res.results[c]["out"] for c in range(N_CORES)],
                         axis=0)
    return out

